# revision 32
# baseline (speedup 1.0000x reference)
"""ADM attention block (B=4, C=512, H=W=64) on 8 TRN2 NeuronCores.

Sharding: core = (b, half) = (core//2, core%2). Data-parallel over batch (4)
x query-halves (2), zero collectives. The query half is selected on the host
by permuting the N axis of x so "my" queries are always columns 0:2048.

v3: out-projection folded into V on the host (vp = W_out @ W_v), so phase 2
has no epilogue matmuls; K computed directly in [c, n] orientation (no PE
transposes / no normalize -- the per-key RMS factor r_j rides in as the
per-partition scale AP of the exp activation); Q section only for the near
half; reciprocals batched per nch-pair; elementwise work split DVE/ACT.
All heavy matmuls fp8e4 perf_mode=DoubleRow (K=256/instr).

Numerics: w_qkv host-scaled by S=16 before the fp8 cast; the post-sqrt eps
is compensated exactly by S*eps. exp carries a -4ln2 bias so fp8 ex stays
below the TRN e4m3 max of 240; the 2^-4 factor cancels in h/den. The
residual path stays f32 end-to-end. Output is produced transposed ([C, NH])
to keep the store DMA contiguous; the host unshards accordingly.
"""

import os
from contextlib import ExitStack

import numpy as np
import ml_dtypes

import concourse.bass as bass
import concourse.mybir as mybir
import concourse.tile as tile
from concourse.bass_utils import run_bass_kernel_spmd

B, C, N = 4, 512, 4096
NH = N // 2
P = 128
O3 = 3 * C             # 1536
NCH = N // P           # 32 n-chunks
QCH = NH // P          # 16 query chunks per core
T = NCH // 2           # 16 j-pairs (DoubleRow contracts 256 keys at once)
ISL = 512              # query i-slice
NISL = NH // ISL       # 4 i-slices
NBLK = 8               # 512-n blocks in phase 1
S = 16.0               # host weight scale for fp8
SG = 16.0              # host scale for the Gram matrix G = W^T W
F32 = mybir.dt.float32
BF16 = mybir.dt.bfloat16
F8 = mybir.dt.float8e4
DR = mybir.MatmulPerfMode.DoubleRow
EXP_BIAS = -2.772588722239781  # -4*ln(2): keeps fp8 ex <= ~15 << 240
RC = float(C) ** -0.5

LAST_RESULT = None

_TPB_ENGINES = (
    mybir.EngineType.PE,
    mybir.EngineType.Activation,
    mybir.EngineType.DVE,
    mybir.EngineType.Pool,
    mybir.EngineType.SP,
)


def _split_waits(nc):
    """walrus on this image rejects >1 sem-wait on a TPB instruction. Hoist
    excess waits onto engine-local NoOps, each carrying one wait."""
    ctr = 0
    for fn in nc.m.functions:
        for blk in fn.blocks:
            new_insts = []
            for inst in blk.instructions:
                si = getattr(inst, "sync_info", None)
                eng = getattr(inst, "engine", None)
                if (
                    si is not None
                    and si.on_wait
                    and len(si.on_wait) > 1
                    and eng in _TPB_ENGINES
                ):
                    for sw in si.on_wait[:-1]:
                        ctr += 1
                        nop = mybir.InstNoOp(
                            name=f"wsplit-{ctr}", engine=eng, ins=[], outs=[],
                            sync_info=mybir.SyncInfo(on_wait=[sw], on_update=[]),
                        )
                        new_insts.append(nop)
                    inst.sync_info = mybir.SyncInfo(
                        on_wait=[si.on_wait[-1]], on_update=si.on_update,
                    )
                new_insts.append(inst)
            blk.instructions[:] = new_insts


def build_graph():
    nc = bass.Bass()

    x_pack_d = nc.declare_dram_parameter("x_pack", [P, 4, N], F8, isOutput=False)
    xn_pack_d = nc.declare_dram_parameter("xn_pack", [P, NCH, C], BF16, isOutput=False)
    w_pack_d = nc.declare_dram_parameter("w_pack", [P, 2, 2, 1024], F8, isOutput=False)
    wk_pack_d = nc.declare_dram_parameter("wk_pack", [P, 2, 2, C], F8, isOutput=False)
    g_pack_d = nc.declare_dram_parameter("g_pack", [P, 2, 2, C], F8, isOutput=False)
    ident_d = nc.declare_dram_parameter("ident_bf", [P, P], BF16, isOutput=False)
    xc_nc = nc.declare_dram_parameter("xc_nc", [C, NH], F32, isOutput=False)
    out_d = nc.declare_dram_parameter("out", [C, NH], F32, isOutput=True)

    with tile.TileContext(nc) as tc, ExitStack() as ctx:
        singles = ctx.enter_context(tc.tile_pool(name="singles", bufs=1))

        g_sb = singles.tile([P, 2, 2, C], F8)
        nc.sync.dma_start(out=g_sb, in_=g_pack_d[:, :, :, :])
        # w/wk/ident DMAs are emitted inside block 0 (after its x/xn loads)
        # so the critical first y-matmul operands transfer first.
        w_sb = singles.tile([P, 2, 2, 1024], F8)
        wk_sb = singles.tile([P, 2, 2, C], F8)
        ident = singles.tile([P, P], BF16)
        ones2 = singles.tile([P, 2, P], F8)
        nc.vector.memset(ones2, 1.0)
        ebias = singles.tile([P, 1], F32)
        nc.vector.memset(ebias, EXP_BIAS)
        seps = singles.tile([P, 1], F32)
        nc.vector.memset(seps, S * 1e-4)

        # persistent attention operands
        big = ctx.enter_context(tc.tile_pool(name="big", bufs=1))
        x_sb = big.tile([P, 4, N], F8)      # fp8 x^T: [c-chunk][n] (keys!)
        q_sb = big.tile([P, 4, NH], F8)     # q_hat^T: [c-chunk][i]
        qk_sb = big.tile([P, 4, NH], F8)    # S*(W_k^T q_hat): [c-chunk][i]
        v_sb = big.tile([P, T, 2, C], F8)   # vp_hat:   [j-pair][plane][c]
        rkc_sb = big.tile([P, NCH], F32)    # r * C^-0.5 per j-chunk

        # ---- phase 1: QKV (fp8 DoubleRow) + RMS + operand builds ----
        with tc.tile_pool(name="xbp", bufs=2) as xbp, \
             tc.tile_pool(name="xnp", bufs=3) as xnp, \
             tc.tile_pool(name="yps", bufs=2, space="PSUM") as yps, \
             tc.tile_pool(name="qvps", bufs=4, space="PSUM") as qvpsp, \
             tc.tile_pool(name="kps", bufs=2, space="PSUM") as kpsp, \
             tc.tile_pool(name="sqp", bufs=2) as sqp, \
             tc.tile_pool(name="rp", bufs=3) as rp, \
             tc.tile_pool(name="qnp", bufs=3) as qnp:

            pending = []  # deferred transpose+copy emitters

            def flush_pending():
                while pending:
                    pending.pop(0)()

            def emit_qk(isl):
                # qk[:, i] = S * W_k^T q_hat[:, i]; scores contract it vs x_sb
                for cc in range(4):
                    qkp = kpsp.tile([P, 512], F32, tag="kp", name="qkp")
                    for c2 in range(2):
                        nc.tensor.matmul(
                            qkp,
                            lhsT=wk_sb[:, c2, :, cc * P:(cc + 1) * P],
                            rhs=q_sb[:, 2 * c2:2 * c2 + 2,
                                     isl * ISL:(isl + 1) * ISL],
                            start=(c2 == 0), stop=(c2 == 1), perf_mode=DR)
                    qdst = qk_sb[:, cc, isl * ISL:(isl + 1) * ISL]
                    if isl < 2 and cc % 2 == 0:
                        nc.scalar.copy(out=qdst, in_=qkp)
                    else:
                        nc.vector.tensor_copy(out=qdst, in_=qkp)

            for blk in range(NBLK):
                near = blk < NBLK // 2
                xblk = x_sb[:, :, blk * 512:(blk + 1) * 512]
                nc.sync.dma_start(
                    out=xblk, in_=x_pack_d[:, :, blk * 512:(blk + 1) * 512])
                ssb = rp.tile([P, 4], F32, tag="ssb")
                rr = rp.tile([P, 4], F32, tag="rr")
                for i2 in range(2):
                    qtiles = {}
                    for i in (2 * i2, 2 * i2 + 1):
                        nch = blk * 4 + i
                        xn_sb = xnp.tile([P, C], BF16, tag="xn_sb")
                        nc.sync.dma_start(out=xn_sb, in_=xn_pack_d[:, nch, :])
                        if blk == 0:
                            # weight loads go after the latency-critical
                            # first x/xn transfers (w as two parallel rings)
                            if i == 0:
                                nc.sync.dma_start(
                                    out=w_sb[:, :, :, 0:512],
                                    in_=w_pack_d[:, :, :, 0:512])
                                nc.sync.dma_start(
                                    out=w_sb[:, :, :, 512:1024],
                                    in_=w_pack_d[:, :, :, 512:1024])
                            elif i == 1:
                                nc.sync.dma_start(
                                    out=wk_sb, in_=wk_pack_d[:, :, :, :])
                            elif i == 2:
                                nc.sync.dma_start(out=ident, in_=ident_d[:, :])
                        xl = xblk[:, :, i * P:(i + 1) * P]
                        y = yps.tile([P, 512], F32, tag="y")
                        for c2 in range(2):
                            nc.tensor.matmul(
                                y, lhsT=xl[:, 2 * c2:2 * c2 + 2, :],
                                rhs=g_sb[:, c2, :, :],
                                start=(c2 == 0), stop=(c2 == 1), perf_mode=DR)
                        if near:
                            qp = qvpsp.tile([P, 512], F32, tag="qvp", name="qp")
                            qtiles[i] = qp
                            for c2 in range(2):
                                nc.tensor.matmul(
                                    qp, lhsT=xl[:, 2 * c2:2 * c2 + 2, :],
                                    rhs=w_sb[:, c2, :, 0:512],
                                    start=(c2 == 0), stop=(c2 == 1), perf_mode=DR)
                        vp = qvpsp.tile([P, 512], F32, tag="qvp", name="vp")
                        qtiles[(i, 'v')] = vp
                        for c2 in range(2):
                            nc.tensor.matmul(
                                vp, lhsT=xl[:, 2 * c2:2 * c2 + 2, :],
                                rhs=w_sb[:, c2, :, 512:1024],
                                start=(c2 == 0), stop=(c2 == 1), perf_mode=DR)
                        sq = sqp.tile([P, 512], BF16, tag="sq")
                        nc.vector.scalar_tensor_tensor(
                            out=sq, in0=y, scalar=1.0, in1=xn_sb,
                            op0=mybir.AluOpType.mult, op1=mybir.AluOpType.mult,
                            accum_out=ssb[:, i:i + 1])
                    # deferred transposes from the previous pair run here on PE
                    flush_pending()
                    # far blocks carry one qk i-slice each (PE has slack there)
                    if not near and i2 == 0:
                        emit_qk(blk - 4)
                    # r chain for the pair (batched [P,2])
                    sl = slice(2 * i2, 2 * i2 + 2)
                    nc.scalar.activation(
                        out=rr[:, sl], in_=ssb[:, sl],
                        func=mybir.ActivationFunctionType.Sqrt,
                        scale=S * S / (O3 * SG))
                    if near:
                        nc.vector.tensor_scalar_add(rr[:, sl], rr[:, sl], S * 1e-4)
                    else:
                        nc.scalar.add(rr[:, sl], rr[:, sl], seps[:, 0:1])
                    nc.vector.reciprocal(rr[:, sl], rr[:, sl])
                    rkc_dst = rkc_sb[:, blk * 4 + 2 * i2: blk * 4 + 2 * i2 + 2]
                    if near:
                        nc.vector.tensor_scalar_mul(rkc_dst, rr[:, sl], RC)
                    else:
                        nc.scalar.mul(rkc_dst, rr[:, sl], RC)
                    for i in (2 * i2, 2 * i2 + 1):
                        nch = blk * 4 + i
                        rsl = rr[:, i:i + 1]
                        vp = qtiles[(i, 'v')]
                        if blk >= 6:
                            # tail blocks: keep ACT free for phase 2's exps
                            nc.vector.tensor_scalar_mul(
                                v_sb[:, nch // 2, nch % 2, :], vp, rsl)
                        else:
                            nc.scalar.activation(
                                out=v_sb[:, nch // 2, nch % 2, :], in_=vp,
                                func=mybir.ActivationFunctionType.Copy, scale=rsl)
                        if near:
                            qp = qtiles[i]
                            qn = qnp.tile([P, 512], BF16, tag="qn")
                            nc.vector.tensor_scalar_mul(qn, qp, rsl)

                            def emit_transpose(qn=qn, nch=nch):
                                tt = kpsp.tile([P, 512], F32, tag="kp")
                                ttb = tt.bitcast(BF16)
                                for cc in range(4):
                                    nc.tensor.transpose(
                                        out=ttb[:, cc * P:(cc + 1) * P],
                                        in_=qn[:, cc * P:(cc + 1) * P],
                                        identity=ident)
                                nc.scalar.copy(
                                    out=q_sb[:, :, nch * P:(nch + 1) * P],
                                    in_=ttb[:, 0:512])
                            pending.append(emit_transpose)
            flush_pending()

        # ---- phase 2: attention (fp8 DoubleRow), hT accumulation ----
        scp = ctx.enter_context(tc.tile_pool(name="scp", bufs=2, space="PSUM"))
        hps = ctx.enter_context(tc.tile_pool(name="hps", bufs=5, space="PSUM"))
        dps = ctx.enter_context(tc.tile_pool(name="dps", bufs=1, space="PSUM"))
        expp = ctx.enter_context(tc.tile_pool(name="expp", bufs=3))
        rdp = ctx.enter_context(tc.tile_pool(name="rdp", bufs=2))
        htp = ctx.enter_context(tc.tile_pool(name="htp", bufs=4))
        xtp = ctx.enter_context(tc.tile_pool(name="xtp", bufs=4))
        obp = ctx.enter_context(tc.tile_pool(name="obp", bufs=4))

        tiles = {}  # isl -> ([h0..h3], den), allocated lazily at first acc

        def scores_exp(isl, t):
            ex = expp.tile([P, 2, 512], F8, tag="ex")
            for pl in range(2):
                j = 2 * t + pl
                sc = scp.tile([P, 512], F32, tag="sc")
                for c2 in range(2):
                    nc.tensor.matmul(
                        sc,
                        lhsT=x_sb[:, 2 * c2:2 * c2 + 2, j * P:(j + 1) * P],
                        rhs=qk_sb[:, 2 * c2:2 * c2 + 2, isl * ISL:(isl + 1) * ISL],
                        start=(c2 == 0), stop=(c2 == 1),
                        perf_mode=DR,
                    )
                nc.scalar.activation(out=ex[:, pl, :], in_=sc,
                                     func=mybir.ActivationFunctionType.Exp,
                                     scale=rkc_sb[:, j:j + 1], bias=ebias)
            return ex

        def acc_h_den(isl, t, ex):
            if isl not in tiles:
                hts = [hps.tile([P, 512], F32, tag="h", name=f"h{cc}")
                       for cc in range(4)]
                den_t = dps.tile([P, 512], F32, tag="den")
                tiles[isl] = (hts, den_t)
            hts, den_ps = tiles[isl]
            nc.tensor.matmul(
                den_ps, lhsT=ones2, rhs=ex,
                start=(t == 0), stop=(t == T - 1), perf_mode=DR,
            )
            for cc in range(4):
                nc.tensor.matmul(
                    hts[cc],
                    lhsT=v_sb[:, t, :, cc * P:(cc + 1) * P],
                    rhs=ex,
                    start=(t == 0), stop=(t == T - 1),
                    perf_mode=DR,
                )

        def epilogue(isl):
            hts, den_ps = tiles.pop(isl)
            rden = rdp.tile([P, 512], F32, tag="rden")
            nc.vector.reciprocal(rden, den_ps)
            for cc in range(4):
                xt_sb = xtp.tile([P, 512], F32, tag="xt_sb")
                nc.sync.dma_start(
                    out=xt_sb,
                    in_=xc_nc[cc * P:(cc + 1) * P, isl * ISL:(isl + 1) * ISL])
                hn = htp.tile([P, 512], BF16, tag="hn")
                nc.vector.tensor_mul(hn, hts[cc], rden)
                ob = obp.tile([P, 512], F32, tag="ob")
                if cc % 2 == 0:
                    nc.gpsimd.tensor_add(ob, hn, xt_sb)
                else:
                    nc.vector.tensor_add(ob, hn, xt_sb)
                nc.sync.dma_start(
                    out=out_d[cc * P:(cc + 1) * P, isl * ISL:(isl + 1) * ISL],
                    in_=ob)

        # flat software pipeline over all (isl, t) pairs: emit scores(t+1)
        # before h/den(t) so the PE computes scores while ACT runs exp; the
        # per-isl epilogue is emitted right after its last h/den lands.
        pairs = [(isl, t) for isl in range(NISL) for t in range(T)]
        prev = None
        for cur in pairs:
            ex_cur = scores_exp(*cur)
            if prev is not None:
                acc_h_den(prev[0], prev[1], ex_prev)
                if prev[1] == T - 1:
                    epilogue(prev[0])
            prev, ex_prev = cur, ex_cur
        acc_h_den(prev[0], prev[1], ex_prev)
        epilogue(prev[0])

    _split_waits(nc)
    return nc


_GRAPH = None


def _f8(a):
    return np.asarray(a, dtype=np.float32).astype(ml_dtypes.float8_e4m3)


def kernel(**inputs):
    global _GRAPH, LAST_RESULT
    x = np.ascontiguousarray(np.asarray(inputs["x"], dtype=np.float32))
    v_qkv = np.asarray(inputs["v_qkv"], dtype=np.float32)
    g_qkv = np.asarray(inputs["g_qkv"], dtype=np.float32)
    v_out = np.asarray(inputs["v_out"], dtype=np.float32)
    g_out = np.asarray(inputs["g_out"], dtype=np.float32)

    # weight norm on host
    w_qkv = (g_qkv[:, None] * v_qkv
             / np.linalg.norm(v_qkv.astype(np.float64), axis=1, keepdims=True)
             ).astype(np.float32)  # [3C, C]
    w_out = (g_out[:, None] * v_out
             / np.linalg.norm(v_out.astype(np.float64), axis=1, keepdims=True)
             ).astype(np.float32)  # [C, C]

    rsqrt2 = np.float32(2.0 ** -0.5)
    # fold the out-projection into V: vp section = rsqrt2 * w_out @ w_v
    w_eff = np.concatenate([
        S * w_qkv[0:C],                          # q
        S * rsqrt2 * (w_out @ w_qkv[2 * C:]),    # vp = W_out W_v
    ], axis=0)  # [2C, C]
    # [128 p, 2 c2, 2 pl, 2C]: w_pack[p,c2,pl,o] = w_eff[o, c2*256+pl*128+p]
    wq = w_eff.T.reshape(2, 2, P, 2 * C)
    w_pack = _f8(np.ascontiguousarray(wq.transpose(2, 0, 1, 3)))
    # wk_pack[p,c2,pl,c] = S*w_k[o = c2*256+pl*128+p, c] (contraction over o)
    wk = (S * w_qkv[C:2 * C]).reshape(2, 2, P, C)
    wk_pack = _f8(np.ascontiguousarray(wk.transpose(2, 0, 1, 3)))
    # Gram matrix for the RMS sum-of-squares (original w_qkv!)
    G = (SG * (w_qkv.T @ w_qkv)).reshape(2, 2, P, C)
    g_pack = _f8(np.ascontiguousarray(G.transpose(2, 0, 1, 3)))
    ident_bf = np.eye(P, dtype=ml_dtypes.bfloat16)

    xt = x.reshape(B, C, N)
    in_maps = []
    for core in range(8):
        b, h = core // 2, core % 2
        if h == 0:
            x_perm = xt[b]
        else:
            x_perm = np.concatenate([xt[b][:, NH:], xt[b][:, :NH]], axis=1)
        x_perm = np.ascontiguousarray(x_perm)
        x_pack = np.ascontiguousarray(
            x_perm.reshape(4, P, N).transpose(1, 0, 2))  # [128, 4cc, N]
        xn_pack = np.ascontiguousarray(
            x_perm.T.reshape(NCH, P, C).transpose(1, 0, 2)
        ).astype(ml_dtypes.bfloat16)  # [128 p, nch, C]: x_perm[c, nch*128+p]
        in_maps.append({
            "x_pack": _f8(x_pack),
            "xn_pack": xn_pack,
            "w_pack": w_pack,
            "wk_pack": wk_pack,
            "g_pack": g_pack,
            "ident_bf": ident_bf,
            "xc_nc": np.ascontiguousarray(x_perm[:, :NH] * rsqrt2),
        })

    if _GRAPH is None:
        _GRAPH = build_graph()

    res = run_bass_kernel_spmd(_GRAPH, in_maps, core_ids=list(range(8)))
    LAST_RESULT = res

    out = np.empty((B, C, N), np.float32)
    for core in range(8):
        b, h = core // 2, core % 2
        out[b][:, h * NH:(h + 1) * NH] = res.results[core]["out"]
    return out.reshape(B, C, 64, 64)


# revision 35
# speedup vs baseline: 1.0266x; 1.0266x over previous
"""ADM attention block (B=4, C=512, H=W=64) on 8 TRN2 NeuronCores.

Sharding: core = (b, half) = (core//2, core%2). Data-parallel over batch (4)
x query-halves (2), zero collectives. The query half is selected on the host
by permuting the N axis of x so "my" queries are always columns 0:2048.

v3: out-projection folded into V on the host (vp = W_out @ W_v), so phase 2
has no epilogue matmuls; K computed directly in [c, n] orientation (no PE
transposes / no normalize -- the per-key RMS factor r_j rides in as the
per-partition scale AP of the exp activation); Q section only for the near
half; reciprocals batched per nch-pair; elementwise work split DVE/ACT.
All heavy matmuls fp8e4 perf_mode=DoubleRow (K=256/instr).

Numerics: w_qkv host-scaled by S=16 before the fp8 cast; the post-sqrt eps
is compensated exactly by S*eps. exp carries a -4ln2 bias so fp8 ex stays
below the TRN e4m3 max of 240; the 2^-4 factor cancels in h/den. The
residual path stays f32 end-to-end. Output is produced transposed ([C, NH])
to keep the store DMA contiguous; the host unshards accordingly.
"""

import os
from contextlib import ExitStack

import numpy as np
import ml_dtypes

import concourse.bass as bass
import concourse.mybir as mybir
import concourse.tile as tile
from concourse.bass_utils import run_bass_kernel_spmd

B, C, N = 4, 512, 4096
NH = N // 2
P = 128
O3 = 3 * C             # 1536
NCH = N // P           # 32 n-chunks
QCH = NH // P          # 16 query chunks per core
T = NCH // 2           # 16 j-pairs (DoubleRow contracts 256 keys at once)
ISL = 512              # query i-slice
NISL = NH // ISL       # 4 i-slices
NBLK = 8               # 512-n blocks in phase 1
S = 16.0               # host weight scale for fp8
SG = 16.0              # host scale for the Gram matrix G = W^T W
F32 = mybir.dt.float32
BF16 = mybir.dt.bfloat16
F8 = mybir.dt.float8e4
DR = mybir.MatmulPerfMode.DoubleRow
EXP_BIAS = -2.772588722239781  # -4*ln(2): keeps fp8 ex <= ~15 << 240
RC = float(C) ** -0.5

LAST_RESULT = None

_TPB_ENGINES = (
    mybir.EngineType.PE,
    mybir.EngineType.Activation,
    mybir.EngineType.DVE,
    mybir.EngineType.Pool,
    mybir.EngineType.SP,
)


def _split_waits(nc):
    """walrus on this image rejects >1 sem-wait on a TPB instruction. Hoist
    excess waits onto engine-local NoOps, each carrying one wait."""
    ctr = 0
    for fn in nc.m.functions:
        for blk in fn.blocks:
            new_insts = []
            for inst in blk.instructions:
                si = getattr(inst, "sync_info", None)
                eng = getattr(inst, "engine", None)
                if (
                    si is not None
                    and si.on_wait
                    and len(si.on_wait) > 1
                    and eng in _TPB_ENGINES
                ):
                    for sw in si.on_wait[:-1]:
                        ctr += 1
                        nop = mybir.InstNoOp(
                            name=f"wsplit-{ctr}", engine=eng, ins=[], outs=[],
                            sync_info=mybir.SyncInfo(on_wait=[sw], on_update=[]),
                        )
                        new_insts.append(nop)
                    inst.sync_info = mybir.SyncInfo(
                        on_wait=[si.on_wait[-1]], on_update=si.on_update,
                    )
                new_insts.append(inst)
            blk.instructions[:] = new_insts


def build_graph():
    nc = bass.Bass()

    x_pack_d = nc.declare_dram_parameter("x_pack", [P, 4, N], F8, isOutput=False)
    xn_pack_d = nc.declare_dram_parameter("xn_pack", [P, NCH, C], BF16, isOutput=False)
    w_pack_d = nc.declare_dram_parameter("w_pack", [P, 2, 2, 1024], F8, isOutput=False)
    wk_pack_d = nc.declare_dram_parameter("wk_pack", [P, 2, 2, C], F8, isOutput=False)
    g_pack_d = nc.declare_dram_parameter("g_pack", [P, 2, 2, C], F8, isOutput=False)
    ident_d = nc.declare_dram_parameter("ident_bf", [P, P], BF16, isOutput=False)
    xc_nc = nc.declare_dram_parameter("xc_nc", [C, NH], F32, isOutput=False)
    out_d = nc.declare_dram_parameter("out", [C, NH], F32, isOutput=True)

    with tile.TileContext(nc) as tc, ExitStack() as ctx:
        singles = ctx.enter_context(tc.tile_pool(name="singles", bufs=1))

        g_sb = singles.tile([P, 2, 2, C], F8)
        nc.sync.dma_start(out=g_sb[:, 0, :, :], in_=g_pack_d[:, 0, :, :])
        nc.sync.dma_start(out=g_sb[:, 1, :, :], in_=g_pack_d[:, 1, :, :])
        # w/wk/ident DMAs are emitted inside block 0 (after its x/xn loads)
        # so the critical first y-matmul operands transfer first.
        w_sb = singles.tile([P, 2, 2, 1024], F8)
        wk_sb = singles.tile([P, 2, 2, C], F8)
        ident = singles.tile([P, P], BF16)
        ones2 = singles.tile([P, 2, P], F8)
        nc.vector.memset(ones2, 1.0)
        ebias = singles.tile([P, 1], F32)
        nc.vector.memset(ebias, EXP_BIAS)
        seps = singles.tile([P, 1], F32)
        nc.vector.memset(seps, S * 1e-4)

        # persistent attention operands
        big = ctx.enter_context(tc.tile_pool(name="big", bufs=1))
        x_sb = big.tile([P, 4, N], F8)      # fp8 x^T: [c-chunk][n] (keys!)
        q_sb = big.tile([P, 4, NH], F8)     # q_hat^T: [c-chunk][i]
        qk_sb = big.tile([P, 4, NH], F8)    # S*(W_k^T q_hat): [c-chunk][i]
        v_sb = big.tile([P, T, 2, C], F8)   # vp_hat:   [j-pair][plane][c]
        rkc_sb = big.tile([P, NCH], F32)    # r * C^-0.5 per j-chunk

        # ---- phase 1: QKV (fp8 DoubleRow) + RMS + operand builds ----
        with tc.tile_pool(name="xbp", bufs=2) as xbp, \
             tc.tile_pool(name="xnp", bufs=3) as xnp, \
             tc.tile_pool(name="yps", bufs=2, space="PSUM") as yps, \
             tc.tile_pool(name="qvps", bufs=4, space="PSUM") as qvpsp, \
             tc.tile_pool(name="kps", bufs=2, space="PSUM") as kpsp, \
             tc.tile_pool(name="sqp", bufs=2) as sqp, \
             tc.tile_pool(name="rp", bufs=3) as rp, \
             tc.tile_pool(name="qnp", bufs=3) as qnp:

            pending = []  # deferred transpose+copy emitters

            def flush_pending():
                while pending:
                    pending.pop(0)()

            def emit_qk(isl):
                # qk[:, i] = S * W_k^T q_hat[:, i]; scores contract it vs x_sb
                for cc in range(4):
                    qkp = kpsp.tile([P, 512], F32, tag="kp", name="qkp")
                    for c2 in range(2):
                        nc.tensor.matmul(
                            qkp,
                            lhsT=wk_sb[:, c2, :, cc * P:(cc + 1) * P],
                            rhs=q_sb[:, 2 * c2:2 * c2 + 2,
                                     isl * ISL:(isl + 1) * ISL],
                            start=(c2 == 0), stop=(c2 == 1), perf_mode=DR)
                    qdst = qk_sb[:, cc, isl * ISL:(isl + 1) * ISL]
                    if cc % 2 == 0:
                        nc.scalar.copy(out=qdst, in_=qkp)
                    else:
                        nc.vector.tensor_copy(out=qdst, in_=qkp)

            for blk in range(NBLK):
                near = blk < NBLK // 2
                xblk = x_sb[:, :, blk * 512:(blk + 1) * 512]
                nc.sync.dma_start(
                    out=xblk, in_=x_pack_d[:, :, blk * 512:(blk + 1) * 512])
                ssb = rp.tile([P, 4], F32, tag="ssb")
                rr = rp.tile([P, 4], F32, tag="rr")
                for i2 in range(2):
                    qtiles = {}
                    for i in (2 * i2, 2 * i2 + 1):
                        nch = blk * 4 + i
                        xn_sb = xnp.tile([P, C], BF16, tag="xn_sb")
                        nc.sync.dma_start(out=xn_sb, in_=xn_pack_d[:, nch, :])
                        if blk == 0:
                            # weight loads go after the latency-critical
                            # first x/xn transfers (w as two parallel rings)
                            if i == 0:
                                nc.sync.dma_start(
                                    out=w_sb[:, :, :, 0:512],
                                    in_=w_pack_d[:, :, :, 0:512])
                                nc.sync.dma_start(
                                    out=w_sb[:, :, :, 512:1024],
                                    in_=w_pack_d[:, :, :, 512:1024])
                            elif i == 1:
                                nc.sync.dma_start(
                                    out=wk_sb, in_=wk_pack_d[:, :, :, :])
                            elif i == 2:
                                nc.sync.dma_start(out=ident, in_=ident_d[:, :])
                        xl = xblk[:, :, i * P:(i + 1) * P]
                        y = yps.tile([P, 512], F32, tag="y")
                        for c2 in range(2):
                            nc.tensor.matmul(
                                y, lhsT=xl[:, 2 * c2:2 * c2 + 2, :],
                                rhs=g_sb[:, c2, :, :],
                                start=(c2 == 0), stop=(c2 == 1), perf_mode=DR)
                        if near:
                            qp = qvpsp.tile([P, 512], F32, tag="qvp", name="qp")
                            qtiles[i] = qp
                            for c2 in range(2):
                                nc.tensor.matmul(
                                    qp, lhsT=xl[:, 2 * c2:2 * c2 + 2, :],
                                    rhs=w_sb[:, c2, :, 0:512],
                                    start=(c2 == 0), stop=(c2 == 1), perf_mode=DR)
                        vp = qvpsp.tile([P, 512], F32, tag="qvp", name="vp")
                        qtiles[(i, 'v')] = vp
                        for c2 in range(2):
                            nc.tensor.matmul(
                                vp, lhsT=xl[:, 2 * c2:2 * c2 + 2, :],
                                rhs=w_sb[:, c2, :, 512:1024],
                                start=(c2 == 0), stop=(c2 == 1), perf_mode=DR)
                        sq = sqp.tile([P, 512], BF16, tag="sq")
                        nc.vector.scalar_tensor_tensor(
                            out=sq, in0=y, scalar=1.0, in1=xn_sb,
                            op0=mybir.AluOpType.mult, op1=mybir.AluOpType.mult,
                            accum_out=ssb[:, i:i + 1])
                    # deferred transposes from the previous pair run here on PE
                    flush_pending()
                    # far blocks carry one qk i-slice each (PE has slack there)
                    if not near and i2 == 0:
                        emit_qk(blk - 4)
                    # r chain for the pair (batched [P,2])
                    sl = slice(2 * i2, 2 * i2 + 2)
                    nc.scalar.activation(
                        out=rr[:, sl], in_=ssb[:, sl],
                        func=mybir.ActivationFunctionType.Sqrt,
                        scale=S * S / (O3 * SG))
                    if near:
                        nc.vector.tensor_scalar_add(rr[:, sl], rr[:, sl], S * 1e-4)
                    else:
                        nc.scalar.add(rr[:, sl], rr[:, sl], seps[:, 0:1])
                    nc.vector.reciprocal(rr[:, sl], rr[:, sl])
                    rkc_dst = rkc_sb[:, blk * 4 + 2 * i2: blk * 4 + 2 * i2 + 2]
                    if near:
                        nc.vector.tensor_scalar_mul(rkc_dst, rr[:, sl], RC)
                    else:
                        nc.scalar.mul(rkc_dst, rr[:, sl], RC)
                    for i in (2 * i2, 2 * i2 + 1):
                        nch = blk * 4 + i
                        rsl = rr[:, i:i + 1]
                        vp = qtiles[(i, 'v')]
                        if blk >= 6 and i % 2 == 0:
                            # tail blocks: split norms so neither DVE nor ACT
                            # gates the last vp-bank release
                            nc.vector.tensor_scalar_mul(
                                v_sb[:, nch // 2, nch % 2, :], vp, rsl)
                        else:
                            nc.scalar.activation(
                                out=v_sb[:, nch // 2, nch % 2, :], in_=vp,
                                func=mybir.ActivationFunctionType.Copy, scale=rsl)
                        if near:
                            qp = qtiles[i]
                            qn = qnp.tile([P, 512], BF16, tag="qn")
                            nc.vector.tensor_scalar_mul(qn, qp, rsl)

                            def emit_transpose(qn=qn, nch=nch):
                                tt = kpsp.tile([P, 512], F32, tag="kp")
                                ttb = tt.bitcast(BF16)
                                for cc in range(4):
                                    nc.tensor.transpose(
                                        out=ttb[:, cc * P:(cc + 1) * P],
                                        in_=qn[:, cc * P:(cc + 1) * P],
                                        identity=ident)
                                nc.scalar.copy(
                                    out=q_sb[:, :, nch * P:(nch + 1) * P],
                                    in_=ttb[:, 0:512])
                            pending.append(emit_transpose)
            flush_pending()

        # ---- phase 2: attention (fp8 DoubleRow), hT accumulation ----
        scp = ctx.enter_context(tc.tile_pool(name="scp", bufs=2, space="PSUM"))
        hps = ctx.enter_context(tc.tile_pool(name="hps", bufs=5, space="PSUM"))
        dps = ctx.enter_context(tc.tile_pool(name="dps", bufs=1, space="PSUM"))
        expp = ctx.enter_context(tc.tile_pool(name="expp", bufs=3))
        rdp = ctx.enter_context(tc.tile_pool(name="rdp", bufs=2))
        htp = ctx.enter_context(tc.tile_pool(name="htp", bufs=4))
        xtp = ctx.enter_context(tc.tile_pool(name="xtp", bufs=4))
        obp = ctx.enter_context(tc.tile_pool(name="obp", bufs=4))

        tiles = {}  # isl -> ([h0..h3], den), allocated lazily at first acc

        def scores_exp(isl, t):
            ex = expp.tile([P, 2, 512], F8, tag="ex")
            for pl in range(2):
                j = 2 * t + pl
                sc = scp.tile([P, 512], F32, tag="sc")
                for c2 in range(2):
                    nc.tensor.matmul(
                        sc,
                        lhsT=x_sb[:, 2 * c2:2 * c2 + 2, j * P:(j + 1) * P],
                        rhs=qk_sb[:, 2 * c2:2 * c2 + 2, isl * ISL:(isl + 1) * ISL],
                        start=(c2 == 0), stop=(c2 == 1),
                        perf_mode=DR,
                    )
                nc.scalar.activation(out=ex[:, pl, :], in_=sc,
                                     func=mybir.ActivationFunctionType.Exp,
                                     scale=rkc_sb[:, j:j + 1], bias=ebias)
            return ex

        def acc_h_den(isl, t, ex):
            if isl not in tiles:
                hts = [hps.tile([P, 512], F32, tag="h", name=f"h{cc}")
                       for cc in range(4)]
                den_t = dps.tile([P, 512], F32, tag="den")
                tiles[isl] = (hts, den_t)
            hts, den_ps = tiles[isl]
            nc.tensor.matmul(
                den_ps, lhsT=ones2, rhs=ex,
                start=(t == 0), stop=(t == T - 1), perf_mode=DR,
            )
            for cc in range(4):
                nc.tensor.matmul(
                    hts[cc],
                    lhsT=v_sb[:, t, :, cc * P:(cc + 1) * P],
                    rhs=ex,
                    start=(t == 0), stop=(t == T - 1),
                    perf_mode=DR,
                )

        def epilogue(isl):
            hts, den_ps = tiles.pop(isl)
            rden = rdp.tile([P, 512], F32, tag="rden")
            nc.vector.reciprocal(rden, den_ps)
            for cc in range(4):
                xt_sb = xtp.tile([P, 512], F32, tag="xt_sb")
                nc.sync.dma_start(
                    out=xt_sb,
                    in_=xc_nc[cc * P:(cc + 1) * P, isl * ISL:(isl + 1) * ISL])
                hn = htp.tile([P, 512], BF16, tag="hn")
                nc.vector.tensor_mul(hn, hts[cc], rden)
                ob = obp.tile([P, 512], F32, tag="ob")
                if cc % 2 == 0:
                    nc.gpsimd.tensor_add(ob, hn, xt_sb)
                else:
                    nc.vector.tensor_add(ob, hn, xt_sb)
                nc.sync.dma_start(
                    out=out_d[cc * P:(cc + 1) * P, isl * ISL:(isl + 1) * ISL],
                    in_=ob)

        # flat software pipeline over all (isl, t) pairs: emit scores(t+1)
        # before h/den(t) so the PE computes scores while ACT runs exp; the
        # per-isl epilogue is emitted right after its last h/den lands.
        pairs = [(isl, t) for isl in range(NISL) for t in range(T)]
        prev = None
        for cur in pairs:
            ex_cur = scores_exp(*cur)
            if prev is not None:
                acc_h_den(prev[0], prev[1], ex_prev)
                if prev[1] == T - 1:
                    epilogue(prev[0])
            prev, ex_prev = cur, ex_cur
        acc_h_den(prev[0], prev[1], ex_prev)
        epilogue(prev[0])

    _split_waits(nc)
    return nc


_GRAPH = None


def _f8(a):
    return np.asarray(a, dtype=np.float32).astype(ml_dtypes.float8_e4m3)


def kernel(**inputs):
    global _GRAPH, LAST_RESULT
    x = np.ascontiguousarray(np.asarray(inputs["x"], dtype=np.float32))
    v_qkv = np.asarray(inputs["v_qkv"], dtype=np.float32)
    g_qkv = np.asarray(inputs["g_qkv"], dtype=np.float32)
    v_out = np.asarray(inputs["v_out"], dtype=np.float32)
    g_out = np.asarray(inputs["g_out"], dtype=np.float32)

    # weight norm on host
    w_qkv = (g_qkv[:, None] * v_qkv
             / np.linalg.norm(v_qkv.astype(np.float64), axis=1, keepdims=True)
             ).astype(np.float32)  # [3C, C]
    w_out = (g_out[:, None] * v_out
             / np.linalg.norm(v_out.astype(np.float64), axis=1, keepdims=True)
             ).astype(np.float32)  # [C, C]

    rsqrt2 = np.float32(2.0 ** -0.5)
    # fold the out-projection into V: vp section = rsqrt2 * w_out @ w_v
    w_eff = np.concatenate([
        S * w_qkv[0:C],                          # q
        S * rsqrt2 * (w_out @ w_qkv[2 * C:]),    # vp = W_out W_v
    ], axis=0)  # [2C, C]
    # [128 p, 2 c2, 2 pl, 2C]: w_pack[p,c2,pl,o] = w_eff[o, c2*256+pl*128+p]
    wq = w_eff.T.reshape(2, 2, P, 2 * C)
    w_pack = _f8(np.ascontiguousarray(wq.transpose(2, 0, 1, 3)))
    # wk_pack[p,c2,pl,c] = S*w_k[o = c2*256+pl*128+p, c] (contraction over o)
    wk = (S * w_qkv[C:2 * C]).reshape(2, 2, P, C)
    wk_pack = _f8(np.ascontiguousarray(wk.transpose(2, 0, 1, 3)))
    # Gram matrix for the RMS sum-of-squares (original w_qkv!)
    G = (SG * (w_qkv.T @ w_qkv)).reshape(2, 2, P, C)
    g_pack = _f8(np.ascontiguousarray(G.transpose(2, 0, 1, 3)))
    ident_bf = np.eye(P, dtype=ml_dtypes.bfloat16)

    xt = x.reshape(B, C, N)
    in_maps = []
    for core in range(8):
        b, h = core // 2, core % 2
        if h == 0:
            x_perm = xt[b]
        else:
            x_perm = np.concatenate([xt[b][:, NH:], xt[b][:, :NH]], axis=1)
        x_perm = np.ascontiguousarray(x_perm)
        x_pack = np.ascontiguousarray(
            x_perm.reshape(4, P, N).transpose(1, 0, 2))  # [128, 4cc, N]
        xn_pack = np.ascontiguousarray(
            x_perm.T.reshape(NCH, P, C).transpose(1, 0, 2)
        ).astype(ml_dtypes.bfloat16)  # [128 p, nch, C]: x_perm[c, nch*128+p]
        in_maps.append({
            "x_pack": _f8(x_pack),
            "xn_pack": xn_pack,
            "w_pack": w_pack,
            "wk_pack": wk_pack,
            "g_pack": g_pack,
            "ident_bf": ident_bf,
            "xc_nc": np.ascontiguousarray(x_perm[:, :NH] * rsqrt2),
        })

    if _GRAPH is None:
        _GRAPH = build_graph()

    res = run_bass_kernel_spmd(_GRAPH, in_maps, core_ids=list(range(8)))
    LAST_RESULT = res

    out = np.empty((B, C, N), np.float32)
    for core in range(8):
        b, h = core // 2, core % 2
        out[b][:, h * NH:(h + 1) * NH] = res.results[core]["out"]
    return out.reshape(B, C, 64, 64)
